# revision 1
# baseline (speedup 1.0000x reference)
"""Trainium2 Bass kernel for DPPDynamicEmbedding (retrieval_knn).

Reference computation (per batch b, N=4096 points in [0,1]^2):
  placed    = (~action_mask) & ~(keepout | probe)                  [N] bool
  d2[i,j]   = |x_i|^2 + |x_j|^2 - 2 x_i.x_j                        [N,N]
  density_i = |{j : placed_j and d2[i,j] < R^2}| / 20              [N]
  proj      = [placed, density] @ W                                [N, 384]
  out       = split(proj, 3) -> (glimpse_key, glimpse_val, logit_key)

Strategy: data-parallel, 2 batches per core on 8 cores.  Per i-block of
128 points (i = p*C + ib, p = SBUF partition, C = N/128):

  - PE computes psum[i, j] = -2 x_i.x_j + masked_sq_j with K=8 fp16
    matmuls.  fp16 hi/lo splitting (x = xh + xl, sq = sqh + sql) keeps
    d2 accurate to ~1e-6 while running single-pass (fp32 matmuls are
    2-pass on TRN2 and much slower).  The j-domain is compacted on the
    host to only placed points, padded to J; pad entries get sq = 32768
    so they can never be inside the radius.
  - K=8 fits a 32-row PE group, so the per-block matmuls cycle through
    array row groups 0/32/64/96 via tile_position; weights and moving
    operands are replicated at partition offsets 0/32/64/96 so the four
    chunk matmuls (and the rotating projection matmul) overlap in the
    array instead of serializing on the ~(219+N)cyc isolated-MM latency.
  - The comparison d2 < R^2  <=>  psum < thresh_i (thresh = R^2 - sq_i)
    is one fused compare+count op per engine per i-block, reading a
    2-bank PSUM tile:
      ACT: Sign(thresh - psum), accum_out S  => count = (S + Wa)/2
      DVE: tensor_scalar is_lt, accum_out C  => count = C
  - Counts (fp32, converted to exact fp16 integers) take a tiny DRAM
    round-trip to become rows [1, N] (p-major flatten == i order) of
    feat^T, written into all four row-group replicas.
  - Projection is one K=8 fp16 matmul per 128 points: lhsT rows =
    [placed, placed, S, S, C, C, 1, 1], rhs rows = hi/lo splits of
    [W0, W1/40, W1/20, (Wa/40) W1] -- count normalization and the
    Sign->count affine fix ride the matmul; the hi/lo W split keeps
    fp32-level accuracy.  PSUM -> SBUF copy (ACT/DVE split; DMA cannot
    read PSUM) then one DMA per two i-blocks into a [N, 384] output.
"""

import numpy as np

import concourse.bass as bass
import concourse.mybir as mybir
import concourse.tile as tile
from concourse import bacc, bass_utils

R2 = 0.16
SCALE = 20.0
BIG = 32768.0          # pad sentinel; must be fp16-exact and >> R2
N_CORES = 8

F32 = mybir.dt.float32
F16 = mybir.dt.float16


def _split16(v):
    """Split fp32 array into (hi, lo) fp16 pair with v ~= hi + lo."""
    hi = v.astype(np.float16)
    lo = (v - hi.astype(np.float32)).astype(np.float16)
    return hi, lo


def _wa_for(J):
    """ACT-side j-width (DVE gets J - wa)."""
    return int(max(512, min(1024, round(J * 0.483 / 64) * 64)))


def _subchunks(w, lim=512):
    out = []
    off = 0
    while off < w:
        out.append((off, min(lim, w - off)))
        off += lim
    return out


def build_program(N, BPC, J, wa, copy_split=4):
    """N points, BPC batches per core, J padded j-count, wa = ACT width."""
    C = N // 128
    NB = N // 128
    wd = J - wa
    # chunk list: (psum kind, psum offset, rhs offset, width, row group)
    chunks = []
    g = 0
    for off, w in _subchunks(wa):
        chunks.append(("a", off, off, w, 32 * (g % 4)))
        g += 1
    for off, w in _subchunks(wd):
        chunks.append(("d", off, wa + off, w, 32 * (g % 4)))
        g += 1

    nc = bacc.Bacc("TRN2", target_bir_lowering=False, debug=False,
                   num_devices=N_CORES)

    xi_d = nc.dram_tensor("xi", [BPC, 8, N], F16, kind="ExternalInput")
    rhs3_d = nc.dram_tensor("rhs3", [BPC, 8, J], F16, kind="ExternalInput")
    th_d = nc.dram_tensor("thresh", [BPC, 128, C], F32, kind="ExternalInput")
    ft_d = nc.dram_tensor("featT", [BPC, 8, N], F16, kind="ExternalInput")
    rw_d = nc.dram_tensor("rhsW", [8, 384], F16, kind="ExternalInput")
    pj_d = nc.dram_tensor("proj", [BPC, N, 384], F32, kind="ExternalOutput")

    with tile.TileContext(nc) as tc:
        with (
            tc.tile_pool(name="const", bufs=BPC) as cpool,
            tc.tile_pool(name="accp", bufs=BPC) as accp,
            tc.tile_pool(name="pa", bufs=2, space="PSUM") as pap,
            tc.tile_pool(name="pdv", bufs=2, space="PSUM") as pdp,
            tc.tile_pool(name="scr_a", bufs=3) as scra,
            tc.tile_pool(name="scr_d", bufs=3) as scrd,
            tc.tile_pool(name="outsb", bufs=8) as outp,
            tc.tile_pool(name="dram", bufs=BPC, space="DRAM") as dram,
            tc.tile_pool(name="w", bufs=1) as wpool,
        ):
            def load_repl(pool, tag, name, dram_ap, rows, cols, eng):
                t = pool.tile([128, cols], F16, tag=tag, name=name)
                eng.dma_start(t[:rows, :], dram_ap)
                engs = ((nc.sync, nc.gpsimd, nc.scalar) if eng is nc.sync
                        else (nc.gpsimd,) * 3)
                for e, grp in zip(engs, (32, 64, 96)):
                    e.dma_start(t[grp:grp + rows, :], t[:rows, :])
                return t

            xi, rhs3, th, ft, acc_a, acc_d, sd_a, sd_d = \
                [], [], [], [], [], [], [], []
            for b in range(BPC):
                eng = nc.sync if b == 0 else nc.gpsimd
                xi.append(load_repl(cpool, "xi", f"xi{b}", xi_d.ap()[b],
                                    8, N, eng))
                rhs3.append(load_repl(cpool, "rhs3", f"rhs3{b}",
                                      rhs3_d.ap()[b], 8, J, eng))
                t = cpool.tile([128, C], F32, tag="th", name=f"th{b}")
                eng.dma_start(t[:], th_d.ap()[b])
                th.append(t)
            rhsW = load_repl(wpool, "rw", "rhsW", rw_d.ap(), 8, 384,
                             nc.gpsimd)
            for b in range(BPC):
                ft.append(load_repl(cpool, "ft", f"ft{b}", ft_d.ap()[b],
                                    8, N, nc.gpsimd))
            for b in range(BPC):
                acc_a.append(accp.tile([128, C], F32, tag="aa", name=f"aa{b}"))
                acc_d.append(accp.tile([128, C], F32, tag="ad", name=f"ad{b}"))
                sd_a.append(dram.tile([4, 128, C], F16, tag="sa",
                                      name=f"sda{b}"))

            # ---- fused phases: counts(b) interleaved with proj(b-1) ----
            def counts_block(b, ib):
                isl = slice(ib * 128, (ib + 1) * 128)
                pa = pap.tile([128, 1024], F32, tag="pa", name=f"pa_{b}_{ib}")
                pd = pdp.tile([128, 1024], F32, tag="pd", name=f"pd_{b}_{ib}")
                for kind, po_, ro, w, grp in chunks:
                    dst = pa if kind == "a" else pd
                    nc.tensor.matmul(
                        dst[:, po_:po_ + w],
                        xi[b][grp:grp + 8, isl],
                        rhs3[b][grp:grp + 8, ro:ro + w],
                        start=True, stop=True,
                        tile_position=(grp, 0))
                sa = scra.tile([128, 1024], F16, tag="sa", name=f"sa_{b}_{ib}")
                nc.scalar.activation(
                    sa[:, :wa], pa[:, :wa],
                    mybir.ActivationFunctionType.Sign,
                    bias=th[b][:, ib:ib + 1], scale=-1.0,
                    accum_out=acc_a[b][:, ib:ib + 1])
                sd = scrd.tile([128, 1024], F16, tag="sd", name=f"sd_{b}_{ib}")
                nc.vector.tensor_scalar(
                    sd[:, :wd], pd[:, :wd],
                    th[b][:, ib:ib + 1], None,
                    op0=mybir.AluOpType.is_lt,
                    op1=mybir.AluOpType.add,
                    accum_out=acc_d[b][:, ib:ib + 1])

            def roundtrip(b):
                # counts: fp32 -> exact fp16 ints -> DRAM -> featT rows
                # scratch rows [S, S, C, C]; p-major flatten == i order.
                a16 = accp.tile([128, C], F16, tag="a16", name=f"a16_{b}")
                nc.scalar.copy(a16[:], acc_a[b][:])
                d16 = accp.tile([128, C], F16, tag="d16", name=f"d16_{b}")
                nc.scalar.copy(d16[:], acc_d[b][:])
                for r, t16 in ((0, a16), (1, a16), (2, d16), (3, d16)):
                    nc.sync.dma_start(sd_a[b][r, :, :], t16[:])
                rows4 = sd_a[b][:, :, :].rearrange("r p c -> r (p c)")
                for e, grp in zip((nc.sync, nc.gpsimd, nc.sync, nc.gpsimd),
                                  (0, 32, 64, 96)):
                    e.dma_start(ft[b][grp + 2:grp + 6, :], rows4)

            def proj_pair(b, ib2):
                osb = outp.tile([128, 768], F32, tag="osb",
                                name=f"osb_{b}_{ib2}")
                pool = pap if (ib2 // 2) % 2 == 0 else pdp
                tag = "pa" if pool is pap else "pd"
                po = pool.tile([128, 1024], F32, tag=tag,
                               name=f"po_{b}_{ib2}")
                for s_ in range(2):
                    ib = ib2 + s_
                    grp = 32 * (ib % 4)
                    nc.tensor.matmul(
                        po[:, s_ * 512:s_ * 512 + 384],
                        ft[b][grp:grp + 8, ib * 128:(ib + 1) * 128],
                        rhsW[grp:grp + 8, :], start=True, stop=True,
                        tile_position=(grp, 0))
                pov = po[:].rearrange("p (s k) -> p s k", s=2)[:, :, :384]
                if (ib2 // 2) % 2 == 0:
                    nc.vector.tensor_copy(
                        osb[:].rearrange("p (s k) -> p s k", s=2), pov)
                else:
                    nc.scalar.copy(
                        osb[:].rearrange("p (s k) -> p s k", s=2), pov)
                dst = pj_d.ap()[b, ib2 * 128:(ib2 + 2) * 128, :] \
                    .rearrange("(s p) k -> p s k", p=128)
                deng = (nc.sync, nc.gpsimd, nc.scalar)[(ib2 // 2) % 3]
                deng.dma_start(
                    dst, osb[:].rearrange("p (s k) -> p s k", s=2))

            for b in range(BPC):
                npairs = 0
                for ib in range(NB):
                    counts_block(b, ib)
                    if b > 0 and ib >= 3 and ib % 2 == 1:
                        proj_pair(b - 1, 2 * npairs)
                        npairs += 1
                        if ib >= NB - 4 and npairs < NB // 2:
                            proj_pair(b - 1, 2 * npairs)
                            npairs += 1
                roundtrip(b)
                if b > 0:
                    while npairs < NB // 2:
                        proj_pair(b - 1, 2 * npairs)
                        npairs += 1
            for ib2 in range(0, NB, 2):
                proj_pair(BPC - 1, ib2)
    nc.compile()
    return nc


def prep_core_inputs(action_mask, keepout, probe, locs, W, J, wa):
    """Host-side prep for one core's batches. Returns in_map dict."""
    BPC, N, _ = locs.shape
    C = N // 128

    placed = (~action_mask) & ~(keepout | probe)          # [BPC, N] bool
    placed_f = placed.astype(np.float32)
    x = locs.astype(np.float32)
    sq = (x ** 2).sum(-1)                                 # [BPC, N]
    thresh = (R2 - sq).astype(np.float32)

    # p-major i-block layout: lhsT col m of block ib <- i = m*C + ib
    m = np.arange(128)
    src = (m[None, :] * C + np.arange(C)[:, None]).reshape(-1)  # pos ib*128+m

    xi = np.zeros((BPC, 8, N), np.float16)
    rhs3 = np.zeros((BPC, 8, J), np.float16)
    featT = np.zeros((BPC, 8, N), np.float16)
    th_pm = np.zeros((BPC, 128, C), np.float32)

    for b in range(BPC):
        x0h, x0l = _split16(x[b, :, 0])
        x1h, x1l = _split16(x[b, :, 1])

        idx = np.nonzero(placed[b])[0]
        np_ = len(idx)
        assert np_ <= J, f"placed count {np_} exceeds J={J}"
        j0h, j0l = _split16(-2.0 * x[b, idx, 0])
        j1h, j1l = _split16(-2.0 * x[b, idx, 1])
        sqh, sql = _split16(sq[b, idx])

        # lhsT rows: [xh0, xh0, xl0, xh1, xh1, xl1, 1, 1]
        xi[b, 0, :] = x0h[src]
        xi[b, 1, :] = x0h[src]
        xi[b, 2, :] = x0l[src]
        xi[b, 3, :] = x1h[src]
        xi[b, 4, :] = x1h[src]
        xi[b, 5, :] = x1l[src]
        xi[b, 6, :] = 1.0
        xi[b, 7, :] = 1.0
        # rhs rows pair to give xh*(-2xh) + xh*(-2xl) + xl*(-2xh)
        # per coord, plus 1*sqh + 1*sql (pads: sq = BIG).
        rhs3[b, 6, :] = BIG
        rhs3[b, 0, :np_] = j0h
        rhs3[b, 1, :np_] = j0l
        rhs3[b, 2, :np_] = j0h
        rhs3[b, 3, :np_] = j1h
        rhs3[b, 4, :np_] = j1l
        rhs3[b, 5, :np_] = j1h
        rhs3[b, 6, :np_] = sqh
        rhs3[b, 7, :np_] = sql
        # featT rows: [pl, pl, S, S, C, C, 1, 1]; S/C filled on device
        featT[b, 0, :] = placed_f[b]
        featT[b, 1, :] = placed_f[b]
        featT[b, 6, :] = 1.0
        featT[b, 7, :] = 1.0

        th_pm[b] = thresh[b].reshape(128, C)

    W = W.astype(np.float32)
    rhsW = np.zeros((8, 384), np.float16)
    rows = [W[0],                         # placed
            W[1] / (2.0 * SCALE),         # S (ACT sign-sum)
            W[1] / SCALE,                 # C (DVE count)
            (wa / (2.0 * SCALE)) * W[1]]  # ones (Sign affine fix)
    for r, v in enumerate(rows):
        h, lo = _split16(v)
        rhsW[2 * r] = h
        rhsW[2 * r + 1] = lo

    return {"xi": xi, "rhs3": rhs3, "thresh": th_pm, "featT": featT,
            "rhsW": rhsW}


_PROGRAM_CACHE = {}


def kernel(action_mask, keepout, probe, locs, W, _trace=False, _tmpdir=None):
    action_mask = np.asarray(action_mask)
    keepout = np.asarray(keepout)
    probe = np.asarray(probe)
    locs = np.asarray(locs, dtype=np.float32)
    W = np.asarray(W, dtype=np.float32)

    B, N = action_mask.shape
    BPC = B // N_CORES

    placed = (~action_mask) & ~(keepout | probe)
    max_placed = int(placed.sum(1).max())
    J = max(1536, ((max_placed + 63) // 64) * 64)
    wa = _wa_for(J)

    key = (N, BPC, J, wa)
    if key not in _PROGRAM_CACHE:
        _PROGRAM_CACHE[key] = build_program(N, BPC, J, wa)
    nc = _PROGRAM_CACHE[key]

    in_maps = []
    for c in range(N_CORES):
        s = slice(c * BPC, (c + 1) * BPC)
        in_maps.append(prep_core_inputs(
            action_mask[s], keepout[s], probe[s], locs[s], W, J, wa))

    res = bass_utils.run_bass_kernel_spmd(
        nc, in_maps, core_ids=list(range(N_CORES)),
        trace=_trace, tmpdir=_tmpdir)

    proj = np.concatenate([res.results[c]["proj"] for c in range(N_CORES)], 0)
    out = (np.ascontiguousarray(proj[:, :, :128]),
           np.ascontiguousarray(proj[:, :, 128:256]),
           np.ascontiguousarray(proj[:, :, 256:384]))
    if _trace:
        return out, res
    return out



# revision 12
# speedup vs baseline: 1.0016x; 1.0016x over previous
"""Trainium2 Bass kernel for DPPDynamicEmbedding (retrieval_knn).

Reference computation (per batch b, N=4096 points in [0,1]^2):
  placed    = (~action_mask) & ~(keepout | probe)                  [N] bool
  d2[i,j]   = |x_i|^2 + |x_j|^2 - 2 x_i.x_j                        [N,N]
  density_i = |{j : placed_j and d2[i,j] < R^2}| / 20              [N]
  proj      = [placed, density] @ W                                [N, 384]
  out       = split(proj, 3) -> (glimpse_key, glimpse_val, logit_key)

Strategy (v2): data-parallel, 2 batches per core on 8 cores, with
host-side spatial pruning of the pair-compare work:

  - The 4096 i-points are sorted into 32 spatial cells of 128 points
    (8 x-strips of 512 by rank, then 4 y-cells of 128 by rank).  Each
    cell becomes one i-block (slot); psum partition p of slot s holds
    point perm2[128*s + p].
  - Per cell: candidates = placed j with dist(center, j) inside the
    annulus [R - maxd - eps, R + maxd + eps).  Placed j fully inside
    (d < R - maxd - eps) are counted on the host into a per-cell n0
    that rides the C-correction row; j fully outside are dropped.
    This cuts the compare width from ~1856 to ~650 per slot.
  - d2 partials come from K=8 fp16 hi/lo matmuls as in v1 (psum =
    -2 x_i.x_j + sq_j, pads get sq = 32768), in 512-wide pieces
    spread over PE array row groups 0/32/64/96 via tile_position.
  - The compare+count d2 < R^2  <=>  psum < thresh_i is ONE engine op
    per slot-part with accum_out: whole slots are assigned to either
    ACT (Sign) or DVE (is_lt) by a greedy load balancer (one
    intercept per slot instead of two); slots wider than 512 use DVE
    (<=1024) or are split across both engines.
  - Counts take the v1 DRAM round-trip to become featT rows [S,S,C,C]
    (p-major flatten == featT column order).  C rows carry
    corr = n0 + wa/2 (host-added via one tensor_tensor per batch),
    which folds both the inner-disk bulk count and the Sign affine
    fix, so rhsW needs no ones rows (K=6 projection).
  - Projection: per 128-point block, one K=6 fp16 matmul
    (featT rows [pl,pl,S,S,C,C] x rhsW hi/lo rows) into a dedicated
    2-buf PSUM pool (no contention with count tiles), PSUM->SBUF
    fp16 copy alternating ACT/DVE, one DMA per block.  Output is
    fp16; the host unpermutes rows and casts to fp32.
  - PSUM: pa 2x[128,512] + pd 2x[128,1024] + proj 2x[128,512]
    = 8 banks exactly.
"""

import numpy as np

import concourse.bass as bass
import concourse.mybir as mybir
import concourse.tile as tile
from concourse import bacc, bass_utils

R = 0.4
R2 = 0.16
SCALE = 20.0
BIG = 32768.0          # pad sentinel; must be fp16-exact and >> R2
N_CORES = 8
EPS = 1e-4             # host pruning margin vs ~1e-6 device d2 error
PAD = 16               # width padding granule

F32 = mybir.dt.float32
F16 = mybir.dt.float16

WA_MAX = 512           # pa psum tile width  (1 bank)
WD_MAX = 1024          # pd psum tile width  (2 banks)

# measured per-op cost model (ns) for the greedy engine balancer
ACT_INT, ACT_SLOPE = 385.0, 1.0 / 1.2
DVE_INT, DVE_SLOPE = 147.0, 1.0 / 0.96


def _split16(v):
    """Split fp32 array into (hi, lo) fp16 pair with v ~= hi + lo."""
    hi = v.astype(np.float16)
    lo = (v - hi.astype(np.float32)).astype(np.float16)
    return hi, lo


def _pad16(w):
    return ((int(w) + PAD - 1) // PAD) * PAD


def assign_engines(widths):
    """widths: [BPC][32] padded slot widths (desc).  Returns
    (wa, wd, wd2): per-slot ACT width, DVE width, and second-DVE
    width (monster slots > WA_MAX+WD_MAX use two DVE ops; the second
    accumulates 2*count into the S column, exact since S carries
    coefficient W1/40)."""
    BPC = len(widths)
    wa = [[0] * len(widths[0]) for _ in range(BPC)]
    wd = [[0] * len(widths[0]) for _ in range(BPC)]
    wd2 = [[0] * len(widths[0]) for _ in range(BPC)]
    loadA = 0.0
    loadD = 2000.0  # DVE's proj-copy share is slightly heavier
    for b in range(BPC):
        for s, W in enumerate(widths[b]):
            if W == 0:
                continue
            assert W <= 2 * WD_MAX, f"slot width {W} > {2*WD_MAX}"
            best = None
            cands = []
            if W <= WA_MAX:
                cands.append(("A", W, 0, 0))
                cands.append(("D", 0, W, 0))
            elif W <= WD_MAX:
                cands.append(("D", 0, W, 0))
            if W <= WA_MAX + WD_MAX:
                # split option across both engines
                lo = max(W - WD_MAX, PAD)
                a_part = min(WA_MAX, max(lo, _pad16(int(0.556 * W) - 127)))
                cands.append(("S", a_part, W - a_part, 0))
            else:
                cands.append(("E", 0, WD_MAX, W - WD_MAX))
            for kind, a_w, d_w, e_w in cands:
                cA = loadA + (ACT_INT + ACT_SLOPE * a_w if a_w else 0.0)
                cD = loadD + (DVE_INT + DVE_SLOPE * d_w if d_w else 0.0)
                cD += DVE_INT + DVE_SLOPE * e_w if e_w else 0.0
                key = (max(cA, cD), cA + cD)
                if best is None or key < best[0]:
                    best = (key, a_w, d_w, e_w, cA, cD)
            _, a_w, d_w, e_w, cA, cD = best
            wa[b][s], wd[b][s], wd2[b][s] = a_w, d_w, e_w
            loadA, loadD = cA, cD
    return wa, wd, wd2


def _pieces(wa_s, wd_s, wd2_s=0):
    """Piece list for one slot: (kind, psum_off, width) tuples."""
    out = []
    if wa_s:
        assert wa_s <= WA_MAX
        out.append(("a", 0, wa_s))
    off = 0
    while off < wd_s:
        w = min(512, wd_s - off)
        out.append(("d", off, w))
        off += w
    off = 0
    while off < wd2_s:
        w = min(512, wd2_s - off)
        out.append(("e", off, w))
        off += w
    return out


def build_program(N, BPC, wa, wd, wd2, n_pieces_max):
    """N points, BPC batches/core, wa/wd/wd2: [BPC][32] engine widths."""
    C = N // 128
    NB = N // 128

    nc = bacc.Bacc("TRN2", target_bir_lowering=False, debug=False,
                   num_devices=N_CORES)

    xi_d = nc.dram_tensor("xi", [BPC, 8, N], F16, kind="ExternalInput")
    rhs_d = nc.dram_tensor("rhsp", [BPC, n_pieces_max, 8, 512], F16,
                           kind="ExternalInput")
    th_d = nc.dram_tensor("thresh", [BPC, 128, C], F32, kind="ExternalInput")
    co_d = nc.dram_tensor("corr", [BPC, 128, C], F32, kind="ExternalInput")
    ft_d = nc.dram_tensor("featT", [BPC, 2, N], F16, kind="ExternalInput")
    rw_d = nc.dram_tensor("rhsW", [8, 384], F16, kind="ExternalInput")
    pj_d = nc.dram_tensor("proj", [BPC, N, 384], F16, kind="ExternalOutput")

    with tile.TileContext(nc) as tc:
        with (
            tc.tile_pool(name="const", bufs=BPC) as cpool,
            tc.tile_pool(name="accp", bufs=BPC) as accp,
            tc.tile_pool(name="pa", bufs=2, space="PSUM") as pap,
            tc.tile_pool(name="pd", bufs=2, space="PSUM") as pdp,
            tc.tile_pool(name="pp", bufs=2, space="PSUM") as ppp,
            tc.tile_pool(name="rhsp", bufs=4) as rhsp,
            tc.tile_pool(name="scr_a", bufs=2) as scra,
            tc.tile_pool(name="scr_d", bufs=2) as scrd,
            tc.tile_pool(name="outsb", bufs=6) as outp,
            tc.tile_pool(name="dram", bufs=BPC, space="DRAM") as dram,
            tc.tile_pool(name="w", bufs=1) as wpool,
        ):
            def load_repl(pool, tag, name, dram_ap, rows, cols, eng):
                t = pool.tile([128, cols], F16, tag=tag, name=name)
                eng.dma_start(t[:rows, :], dram_ap)
                engs = ((nc.sync, nc.gpsimd, nc.scalar) if eng is nc.sync
                        else (nc.gpsimd,) * 3)
                for e, grp in zip(engs, (32, 64, 96)):
                    e.dma_start(t[grp:grp + rows, :], t[:rows, :])
                return t

            xi, th, co, ft, acc_a, acc_d, sd_a = [], [], [], [], [], [], []
            for b in range(BPC):
                eng = nc.sync if b == 0 else nc.gpsimd
                xi.append(load_repl(cpool, "xi", f"xi{b}", xi_d.ap()[b],
                                    8, N, eng))
                t = cpool.tile([128, C], F32, tag="th", name=f"th{b}")
                eng.dma_start(t[:], th_d.ap()[b])
                th.append(t)
                t = cpool.tile([128, C], F32, tag="co", name=f"co{b}")
                eng.dma_start(t[:], co_d.ap()[b])
                co.append(t)
            rhsW = load_repl(wpool, "rw", "rhsW", rw_d.ap(), 8, 384,
                             nc.gpsimd)
            for b in range(BPC):
                ft.append(load_repl(cpool, "ft", f"ft{b}", ft_d.ap()[b],
                                    2, N, nc.gpsimd))
            sc2 = []
            for b in range(BPC):
                ta = accp.tile([128, C], F32, tag="aa", name=f"aa{b}")
                td = accp.tile([128, C], F32, tag="ad", name=f"ad{b}")
                nc.gpsimd.memset(ta[:], 0.0)
                nc.gpsimd.memset(td[:], 0.0)
                acc_a.append(ta)
                acc_d.append(td)
                ts = accp.tile([128, C], F32, tag="sc2", name=f"sc2{b}")
                nc.gpsimd.memset(ts[:], 1.0)
                for s in range(C):
                    if wd2[b][s]:
                        nc.gpsimd.memset(ts[:, s:s + 1], 2.0)
                sc2.append(ts)
                sd_a.append(dram.tile([4, 128, C], F16, tag="sa",
                                      name=f"sda{b}"))

            piece_ctr = [0]         # global PE group rotation
            piece_idx = [0, 0]      # per-batch rhs_d piece cursor

            def counts_slot(b, s):
                pcs = _pieces(wa[b][s], wd[b][s], wd2[b][s])
                if not pcs:
                    return
                pa_t = pd_t = pe_t = None
                if wa[b][s]:
                    pa_t = pap.tile([128, WA_MAX], F32, tag="pa",
                                    name=f"pa_{b}_{s}")
                if wd[b][s]:
                    pd_t = pdp.tile([128, WD_MAX], F32, tag="pd",
                                    name=f"pd_{b}_{s}")
                if wd2[b][s]:
                    pe_t = pdp.tile([128, WD_MAX], F32, tag="pd",
                                    name=f"pe_{b}_{s}")
                for kind, off, w in pcs:
                    pidx = piece_idx[b]
                    piece_idx[b] += 1
                    grp = 32 * (piece_ctr[0] % 4)
                    piece_ctr[0] += 1
                    rt = rhsp.tile([128, 512], F16, tag="rt",
                                   name=f"rt_{b}_{s}_{pidx}")
                    deng = nc.sync if (pidx % 2 == 0) else nc.gpsimd
                    deng.dma_start(rt[grp:grp + 8, :w],
                                   rhs_d.ap()[b, pidx, :, :w])
                    dst = {"a": pa_t, "d": pd_t, "e": pe_t}[kind]
                    isl = slice(s * 128, (s + 1) * 128)
                    nc.tensor.matmul(
                        dst[:, off:off + w],
                        xi[b][grp:grp + 8, isl],
                        rt[grp:grp + 8, :w],
                        start=True, stop=True,
                        tile_position=(grp, 0))
                if wa[b][s]:
                    sa = scra.tile([128, WA_MAX], F16, tag="sa",
                                   name=f"sa_{b}_{s}")
                    nc.scalar.activation(
                        sa[:, :wa[b][s]], pa_t[:, :wa[b][s]],
                        mybir.ActivationFunctionType.Sign,
                        bias=th[b][:, s:s + 1], scale=-1.0,
                        accum_out=acc_a[b][:, s:s + 1])
                if wd[b][s]:
                    sd = scrd.tile([128, WD_MAX], F16, tag="sd",
                                   name=f"sd_{b}_{s}")
                    nc.vector.tensor_scalar(
                        sd[:, :wd[b][s]], pd_t[:, :wd[b][s]],
                        th[b][:, s:s + 1], None,
                        op0=mybir.AluOpType.is_lt,
                        op1=mybir.AluOpType.add,
                        accum_out=acc_d[b][:, s:s + 1])
                if wd2[b][s]:
                    # overflow count into the S column; the roundtrip
                    # doubles it via sc2 (S coefficient = W1/40)
                    sd2 = scrd.tile([128, WD_MAX], F16, tag="sd",
                                    name=f"sd2_{b}_{s}")
                    nc.vector.tensor_scalar(
                        sd2[:, :wd2[b][s]], pe_t[:, :wd2[b][s]],
                        th[b][:, s:s + 1], None,
                        op0=mybir.AluOpType.is_lt,
                        op1=mybir.AluOpType.add,
                        accum_out=acc_a[b][:, s:s + 1])

            def roundtrip(b):
                # counts: fp32 -> exact fp16 ints -> DRAM -> featT rows
                # S rows raw; C rows get corr = n0 + wa/2 added.
                a16 = accp.tile([128, C], F16, tag="a16", name=f"a16_{b}")
                nc.vector.tensor_mul(a16[:], acc_a[b][:], sc2[b][:])
                d16 = accp.tile([128, C], F16, tag="d16", name=f"d16_{b}")
                nc.vector.tensor_add(d16[:], acc_d[b][:], co[b][:])
                for r, t16 in ((0, a16), (1, a16), (2, d16), (3, d16)):
                    nc.sync.dma_start(sd_a[b][r, :, :], t16[:])
                rows4 = sd_a[b][:, :, :].rearrange("r p c -> r (p c)")
                for e, grp in zip((nc.sync, nc.gpsimd, nc.sync, nc.gpsimd),
                                  (0, 32, 64, 96)):
                    e.dma_start(ft[b][grp + 2:grp + 6, :], rows4)

            def proj_block(b, k):
                grp = 32 * (k % 4)
                po = ppp.tile([128, 512], F32, tag="pp", name=f"po_{b}_{k}")
                nc.tensor.matmul(
                    po[:, :384],
                    ft[b][grp:grp + 6, k * 128:(k + 1) * 128],
                    rhsW[grp:grp + 6, :], start=True, stop=True,
                    tile_position=(grp, 0))
                osb = outp.tile([128, 384], F16, tag="osb",
                                name=f"osb_{b}_{k}")
                if k % 2 == 0:
                    nc.vector.tensor_copy(osb[:], po[:, :384])
                else:
                    nc.scalar.copy(osb[:], po[:, :384])
                deng = (nc.sync, nc.gpsimd, nc.scalar)[k % 3]
                deng.dma_start(pj_d.ap()[b, k * 128:(k + 1) * 128, :],
                               osb[:])

            for b in range(BPC):
                nblk = 0
                for s in range(NB):
                    counts_slot(b, s)
                    if b > 0 and s >= 2:
                        proj_block(b - 1, nblk)
                        nblk += 1
                roundtrip(b)
                if b > 0:
                    while nblk < NB:
                        proj_block(b - 1, nblk)
                        nblk += 1
            for k in range(NB):
                proj_block(BPC - 1, k)
    nc.compile()
    return nc


def prep_core_inputs(action_mask, keepout, probe, locs, W,
                     wa, wd, wd2, n_pieces_max, cells_all, cand_all, n0_all):
    """Host-side prep for one core's batches. Returns (in_map, perms)."""
    BPC, N, _ = locs.shape
    C = N // 128

    placed = (~action_mask) & ~(keepout | probe)          # [BPC, N] bool
    placed_f = placed.astype(np.float32)
    x = locs.astype(np.float32)
    sq = (x ** 2).sum(-1)                                 # [BPC, N]

    xi = np.zeros((BPC, 8, N), np.float16)
    rhs = np.zeros((BPC, n_pieces_max, 8, 512), np.float16)
    rhs[:, :, 6, :] = BIG
    featT = np.zeros((BPC, 2, N), np.float16)
    th_pm = np.zeros((BPC, 128, C), np.float32)
    co_pm = np.zeros((BPC, 128, C), np.float32)
    perms = []

    for b in range(BPC):
        cells = cells_all[b]          # list of 32 arrays of 128 indices
        cands = cand_all[b]           # list of 32 candidate index arrays
        n0s = n0_all[b]               # list of 32 ints
        perm2 = np.concatenate(cells)
        perms.append(perm2)

        x0h, x0l = _split16(x[b, :, 0])
        x1h, x1l = _split16(x[b, :, 1])
        # xi column j corresponds to point perm2[j]
        xi[b, 0, :] = x0h[perm2]
        xi[b, 1, :] = x0h[perm2]
        xi[b, 2, :] = x0l[perm2]
        xi[b, 3, :] = x1h[perm2]
        xi[b, 4, :] = x1h[perm2]
        xi[b, 5, :] = x1l[perm2]
        xi[b, 6, :] = 1.0
        xi[b, 7, :] = 1.0

        # thresh/corr: psum partition p of slot s = point perm2[128s+p]
        thr = (R2 - sq[b][perm2]).astype(np.float32)      # sorted order
        th_pm[b] = thr.reshape(C, 128).T                  # [p, s]

        pidx = 0
        for s in range(C):
            idx = cands[s]
            j0h, j0l = _split16(-2.0 * x[b, idx, 0])
            j1h, j1l = _split16(-2.0 * x[b, idx, 1])
            sqh, sql = _split16(sq[b, idx])
            rows = np.stack([j0h, j0l, j0h, j1h, j1l, j1h, sqh, sql])
            np_ = len(idx)
            off = 0
            for kind, poff, w in _pieces(wa[b][s], wd[b][s], wd2[b][s]):
                take = rows[:, off:off + w]
                rhs[b, pidx, :, :take.shape[1]] = take
                off += w
                pidx += 1
            assert off >= np_, f"slot {s}: {np_} candidates > width {off}"
            co_pm[b, :, s] = n0s[s] + wa[b][s] / 2.0

        # featT rows [placed, placed] in featT-column order q = p*C + s
        # -> point perm2[128*(q%C) + q//C]
        q = np.arange(N)
        fcol_point = perm2[128 * (q % C) + q // C]
        featT[b, 0, :] = placed_f[b][fcol_point]
        featT[b, 1, :] = featT[b, 0, :]

    W = W.astype(np.float32)
    rhsW = np.zeros((8, 384), np.float16)
    rows = [W[0],                         # placed
            W[1] / (2.0 * SCALE),         # S (ACT sign-sum)
            W[1] / SCALE]                 # C' = C + n0 + wa/2
    for r, v in enumerate(rows):
        h, lo = _split16(v)
        rhsW[2 * r] = h
        rhsW[2 * r + 1] = lo

    return ({"xi": xi, "rhsp": rhs, "thresh": th_pm, "corr": co_pm,
             "featT": featT, "rhsW": rhsW}, perms)


def spatial_cells(x):
    """Sort N points into 32 equal-count cells (8 x-strips x 4 y)."""
    N = x.shape[0]
    order = np.argsort(x[:, 0], kind="stable")
    cells = []
    for sxi in range(8):
        strip = order[sxi * (N // 8):(sxi + 1) * (N // 8)]
        ys = strip[np.argsort(x[strip, 1], kind="stable")]
        for cy in range(4):
            cells.append(ys[cy * 128:(cy + 1) * 128])
    return cells


def plan_batch(x, placed):
    """Per batch: cells sorted desc by candidate count, candidate lists,
    inner counts.  Returns (cells, cands, n0s, widths)."""
    cells = spatial_cells(x)
    pl_idx = np.nonzero(placed)[0]
    P = x[pl_idx].astype(np.float64)
    entries = []
    for cell in cells:
        pts = x[cell].astype(np.float64)
        ctr = (pts.min(0) + pts.max(0)) / 2
        maxd = np.sqrt(((pts - ctr) ** 2).sum(1)).max()
        d = np.sqrt(((P - ctr) ** 2).sum(1))
        inner = d < (R - maxd - EPS)
        outer = d >= (R + maxd + EPS)
        cand = pl_idx[~inner & ~outer]
        entries.append((cell, cand, int(inner.sum())))
    entries.sort(key=lambda e: -len(e[1]))
    cells = [e[0] for e in entries]
    cands = [e[1] for e in entries]
    n0s = [e[2] for e in entries]
    widths = [len(c) for c in cands]
    return cells, cands, n0s, widths


_PROGRAM_CACHE = {}


def kernel(action_mask, keepout, probe, locs, W, _trace=False, _tmpdir=None):
    action_mask = np.asarray(action_mask)
    keepout = np.asarray(keepout)
    probe = np.asarray(probe)
    locs = np.asarray(locs, dtype=np.float32)
    W = np.asarray(W, dtype=np.float32)

    B, N = action_mask.shape
    BPC = B // N_CORES
    C = N // 128

    placed = (~action_mask) & ~(keepout | probe)

    # host planning: spatial cells + candidate lists per (core, batch)
    plans = [[None] * BPC for _ in range(N_CORES)]
    for c in range(N_CORES):
        for b in range(BPC):
            g = c * BPC + b
            plans[c][b] = plan_batch(locs[g], placed[g])

    # global slot widths = max across cores (slots sorted desc per core)
    widths = [[0] * C for _ in range(BPC)]
    for b in range(BPC):
        for s in range(C):
            widths[b][s] = _pad16(max(plans[c][b][3][s]
                                      for c in range(N_CORES)))
    wa, wd, wd2 = assign_engines(widths)
    n_pieces = max(sum(len(_pieces(wa[b][s], wd[b][s], wd2[b][s]))
                       for s in range(C)) for b in range(BPC))
    n_pieces = max(n_pieces, 1)

    key = (N, BPC, tuple(map(tuple, wa)), tuple(map(tuple, wd)),
           tuple(map(tuple, wd2)))
    if key not in _PROGRAM_CACHE:
        _PROGRAM_CACHE[key] = build_program(N, BPC, wa, wd, wd2, n_pieces)
    nc = _PROGRAM_CACHE[key]

    in_maps = []
    perm_all = []
    for c in range(N_CORES):
        s = slice(c * BPC, (c + 1) * BPC)
        cells_all = [plans[c][b][0] for b in range(BPC)]
        cand_all = [plans[c][b][1] for b in range(BPC)]
        n0_all = [plans[c][b][2] for b in range(BPC)]
        in_map, perms = prep_core_inputs(
            action_mask[s], keepout[s], probe[s], locs[s], W,
            wa, wd, wd2, n_pieces, cells_all, cand_all, n0_all)
        in_maps.append(in_map)
        perm_all.append(perms)

    res = bass_utils.run_bass_kernel_spmd(
        nc, in_maps, core_ids=list(range(N_CORES)),
        trace=_trace, tmpdir=_tmpdir)

    # un-permute: output row q of batch (c, b) is point
    # perm2[128*(q%C) + q//C]
    q = np.arange(N)
    row2sorted = 128 * (q % C) + q // C
    out = np.empty((B, N, 384), np.float32)
    for c in range(N_CORES):
        pj = res.results[c]["proj"]                       # [BPC, N, 384] f16
        for b in range(BPC):
            dst_rows = perm_all[c][b][row2sorted]
            out[c * BPC + b, dst_rows] = pj[b].astype(np.float32)

    outs = (np.ascontiguousarray(out[:, :, :128]),
            np.ascontiguousarray(out[:, :, 128:256]),
            np.ascontiguousarray(out[:, :, 256:384]))
    if _trace:
        return outs, res
    return outs
